# revision 4
# baseline (speedup 1.0000x reference)
"""ConsistencyLoss kernel for 8 Trainium2 NeuronCores.

Math (per reference):
  For view1: sim = cos_sim_pairwise(y1, z2) [B,N,N]; mask from grid distances;
  loss_v = sum(sim*mask)/sum(mask); out = -(loss_1 + loss_2), N = 28*28 = 784.

Strategy (data-parallel over batch, 8 batches/core x 8 cores):
  Host prep:
    - Grids are separable: pairwise dist^2 D2[n,m] = Dy2[i(n),i'(m)] +
      Dx2[j(n),j'(m)] from two [28,28] tables per batch.
    - n tiled as 7x112 (4 image rows per tile, exact).  The mask band per
      tile fits in a WW-image-row window of m (WW computed from data,
      typically 6 -> 168 moving columns); host computes the window start per
      (batch, tile).
    - 1/||y_n|| folded into y features, 1/||z_m|| folded into z features
      (host, fp32), then features packed to fp16 in one dram tensor
      [BPC, 128, 2(view pair), 2(y|z), 2(c-half), N]; two DMAs per batch so
      view1 compute overlaps view2's feature transfer.
    - Mask counts (denominators) computed on host with bit-identical fp32
      add+compare to the device mask test.
  Device per (batch, tile):
    - Pool/GpSimd: d2 tile [112, WW*28] = dyw ⊕ dxt (broadcast add, SBUF).
    - PE: num_v = y_v^T @ z_v windowed (fp16 in, fp32 PSUM accum over the
      two 128-channel halves); window offset is a runtime PE register
      (bass.ds), 7 offsets loaded per batch in one TensorLoad.
    - DVE: scalar_tensor_tensor (d2 <= thr_v) * num_v with per-partition
      accumulation into ms[v][:, b*7+k].
    - ms accumulators DMA'd raw to HBM at the end (short tail); host does
      the final reduction.
  Host finish: sum the 8 cores' ms, divide by host counts.

The timeline is paced by the feature DMA stream (~36.4us of the ~45us
total); DVE masked-accumulate is the secondary leg (~34us), Pool builds
~24us, PE ~17us.
"""

import sys

sys.path.insert(0, "/opt/trn_rl_repo")

import numpy as np

import concourse.bass as bass
import concourse.mybir as mybir
import concourse.tile as tile
from concourse import bacc
from concourse.bass import broadcast_tensor_aps
from concourse.bass_utils import run_bass_kernel_spmd

B, C, H, W = 64, 256, 28, 28
N = H * W  # 784
NCORES = 8
BPC = B // NCORES  # batches per core
NT = 7  # n tiles of 112 partitions (4 image rows each)
TP = 112  # partitions per tile
THR = 0.7

F32 = mybir.dt.float32
F16 = mybir.dt.float16
I32 = mybir.dt.int32
ALU = mybir.AluOpType
ENG = mybir.EngineType

_COMPILED = {}


def _build_nc(WW=6):
    WWC = WW * 28  # window columns in m
    nc = bacc.Bacc("TRN2", debug=False, num_devices=NCORES)

    # features: [b, p, pr, t, cc, n], pair pr0=(ay1,bz2), pr1=(ay2,bz1)
    feats = nc.dram_tensor("feats", [BPC, 128, 2, 2, 2, N], F16, kind="ExternalInput")
    # small pack: [p(112), b, 7*WW dyw | 28 dxt | 2 thr]
    SX = NT * WW + 28 + 2
    small = nc.dram_tensor("small", [TP, BPC, SX], F32, kind="ExternalInput")
    woff = nc.dram_tensor("woff", [1, BPC * NT], I32, kind="ExternalInput")
    NG = 3  # stt groups per batch (tile triples)
    out = nc.dram_tensor("out", [128, 2, BPC * NG], F32, kind="ExternalOutput")

    with tile.TileContext(nc) as tc:
        with (
            tc.tile_pool(name="feat", bufs=4) as feat_pool,
            tc.tile_pool(name="small", bufs=1) as sm_pool,
            tc.tile_pool(name="d2", bufs=7) as d2_pool,
            tc.tile_pool(name="scr", bufs=3) as scr_pool,
            tc.tile_pool(name="accum", bufs=1) as acc_pool,
            tc.tile_pool(name="psum", bufs=5, space="PSUM") as psum_pool,
        ):
            fts = []
            for _ in range(2):
                ft = feat_pool.tile([128, 2, 2, 2, N], F16, tag="ft")
                fts.append(ft)
            nc.sync.dma_start(fts[0][:, 0, :, :, :], feats[0, :, 0])
            sm_t = sm_pool.tile([TP, BPC, SX], F32)
            nc.sync.dma_start(sm_t[:, :, :], small[:, :, :])
            woff_t = sm_pool.tile([1, BPC * NT], I32)
            nc.sync.dma_start(woff_t[:, :], woff[:, :])
            for cc in (0, 1):
                nc.sync.dma_start(fts[0][:, 1, :, cc, :], feats[0, :, 1, :, cc])
            nc.sync.dma_start(fts[1][:, 0, :, :, :], feats[1, :, 0])
            for cc in (0, 1):
                nc.sync.dma_start(fts[1][:, 1, :, cc, :], feats[1, :, 1, :, cc])

            ms = []
            for v in (0, 1):
                m = acc_pool.tile([128, BPC * NG], F32, tag=f"ms{v}")
                nc.vector.memset(m[:, :], 0.0)
                ms.append(m)

            regsets = [
                [nc.alloc_register(ENG.PE, f"w{s}_{k}") for k in range(NT)]
                for s in (0, 1)
            ]

            for b in range(BPC):
                if b < 2:
                    ft = fts[b]
                else:
                    ft = feat_pool.tile([128, 2, 2, 2, N], F16, tag="ft")
                    nc.sync.dma_start(ft[:, 0, :, :, :], feats[b, :, 0])
                    for cc in (0, 1):
                        nc.sync.dma_start(ft[:, 1, :, cc, :], feats[b, :, 1, :, cc])

                regs = regsets[b % 2]
                nc.tensor.load(regs, woff_t[0:1, b * NT : (b + 1) * NT])
                wvs = [
                    nc.snap(
                        bass.RegisterHandles([regs[k]]),
                        donate=True,
                        min_val=0,
                        max_val=(28 - WW) * 28,
                    )
                    for k in range(NT)
                ]

                # tile triples share one PSUM bank (3*168 fp32 = 2016B <= 2KB)
                # and one d2 tile, so each masked-accumulate is a single wide
                # stt instead of three.
                GROUPS = ((6,), (0, 1, 2), (3, 4, 5))
                d2s = []
                for g, ks in enumerate(GROUPS):
                    gw = len(ks)
                    d2 = d2_pool.tile([TP, 3, WWC], F32, tag="d2")
                    i0, i1 = broadcast_tensor_aps(
                        sm_t[:, b, ks[0] * WW : (ks[0] + gw) * WW, None],
                        sm_t[:, b, None, NT * WW : NT * WW + 28],
                    )
                    nc.gpsimd.tensor_tensor(
                        d2[:, 0:gw, :].rearrange("q g (a c) -> q (g a) c", a=WW),
                        i0,
                        i1,
                        ALU.add,
                    )
                    d2s.append(d2)

                for v in (0, 1):
                    for g, ks in enumerate(GROUPS):
                        gw = len(ks)
                        num = psum_pool.tile([TP, 3, WWC], F32, tag="num")
                        for j, k in enumerate(ks):
                            for cc in (0, 1):
                                nc.tensor.matmul(
                                    num[:, j, :],
                                    ft[:, v, 0, cc, k * TP : (k + 1) * TP],
                                    ft[:, v, 1, cc, bass.ds(wvs[k], WWC)],
                                    start=(cc == 0),
                                    stop=(cc == 1),
                                )
                        scr = scr_pool.tile([TP, 3 * WWC], F32, tag="scr")
                        col = b * NG + g
                        nc.vector.scalar_tensor_tensor(
                            out=scr[:, 0 : gw * WWC],
                            in0=d2s[g][:, 0:gw, :],
                            scalar=sm_t[:, b, NT * WW + 28 + v : NT * WW + 29 + v],
                            in1=num[:, 0:gw, :],
                            op0=ALU.is_le,
                            op1=ALU.mult,
                            accum_out=ms[v][0:TP, col : col + 1],
                        )

            nc.scalar.dma_start(out[:, 0, :], ms[0][:, :])
            nc.sync.dma_start(out[:, 1, :], ms[1][:, :])

    nc.compile()
    return nc


def _get_nc(WW):
    if WW not in _COMPILED:
        _COMPILED[WW] = _build_nc(WW)
    return _COMPILED[WW]


def _prep_host(y1, y2, z1, z2, view1_grid, view2_grid):
    """Host-side prep: separable distance tables, norms, counts, shards."""
    y1f = y1.reshape(B, C, N)
    y2f = y2.reshape(B, C, N)
    z1f = z1.reshape(B, C, N)
    z2f = z2.reshape(B, C, N)

    # --- separable grid tables ------------------------------------------
    g1y = view1_grid[:, 0, :, 0]  # [B, 28]
    g1x = view1_grid[:, 1, 0, :]
    g2y = view2_grid[:, 0, :, 0]
    g2x = view2_grid[:, 1, 0, :]
    if not (
        np.array_equal(view1_grid[:, 0], np.broadcast_to(g1y[:, :, None], (B, H, W)))
        and np.array_equal(view1_grid[:, 1], np.broadcast_to(g1x[:, None, :], (B, H, W)))
        and np.array_equal(view2_grid[:, 0], np.broadcast_to(g2y[:, :, None], (B, H, W)))
        and np.array_equal(view2_grid[:, 1], np.broadcast_to(g2x[:, None, :], (B, H, W)))
    ):
        raise RuntimeError("grids are not separable; unsupported input")

    dy = g1y[:, :, None] - g2y[:, None, :]  # fp32 [B,28,28]
    dx = g1x[:, :, None] - g2x[:, None, :]
    dy2 = dy * dy
    dx2 = dx * dx

    v1bin = np.linalg.norm(view1_grid[..., 1, 1] - view1_grid[..., 0, 0], axis=-1)
    v2bin = np.linalg.norm(view2_grid[..., 1, 1] - view2_grid[..., 0, 0], axis=-1)
    t2 = np.empty((B, 2), np.float32)
    t2[:, 0] = ((THR * v1bin.astype(np.float64)) ** 2).astype(np.float32)
    t2[:, 1] = ((THR * v2bin.astype(np.float64)) ** 2).astype(np.float32)

    # --- per-(batch, tile) windows of valid i' --------------------------
    tmax2 = np.maximum(t2[:, 0], t2[:, 1]).astype(np.float64) * (1 + 1e-6)  # [B]
    w0 = np.zeros((B, NT), np.int32)
    widths = np.zeros((B, NT), np.int64)
    for k in range(NT):
        sub_min = dy2[:, 4 * k : 4 * k + 4, :].min(axis=1)  # [B, 28]
        valid = sub_min <= tmax2[:, None]
        any_valid = valid.any(axis=1)
        first = np.argmax(valid, axis=1)
        last = 27 - np.argmax(valid[:, ::-1], axis=1)
        widths[:, k] = np.where(any_valid, last - first + 1, 1)
        w0[:, k] = np.where(any_valid, first, 0)
    WW = max(6, int(widths.max()))
    if WW > 28:
        raise RuntimeError("mask window exceeds image; unsupported input")
    w0 = np.minimum(w0, 28 - WW).astype(np.int32)

    # dyw[b, p(112), k, a] = dy2[b, 4k + p//28, w0[b,k]+a]
    iidx = np.arange(TP) // 28  # [112] image row within tile
    cols = w0[:, :, None] + np.arange(WW)[None, None, :]  # [B, NT, WW]
    dyw = dy2[
        np.arange(B)[:, None, None, None],
        (iidx[None, :, None, None] + 4 * np.arange(NT)[None, None, :, None]),
        cols[:, None, :, :],
    ]  # [B, 112, NT, WW]
    woff = (w0 * 28).astype(np.int32).reshape(B, NT)

    # dxt[b, p, c] = dx2[b, p%28, c]
    dxt = dx2[:, np.tile(np.arange(28), 4), :]  # [B, 112, 28]

    # --- mask counts (bit-identical fp32 add + compare as device) -------
    counts = np.zeros(2, np.int64)
    for b in range(B):
        d2b = dy2[b][:, None, :, None] + dx2[b][None, :, None, :]  # fp32
        counts[0] += int((d2b <= t2[b, 0]).sum())
        counts[1] += int((d2b <= t2[b, 1]).sum())

    # --- norms (both sides folded on host) ------------------------------
    def rnorm(a):
        n = np.sqrt(np.einsum("bcn,bcn->bn", a, a, dtype=np.float32))
        return 1.0 / np.maximum(n, np.float32(1e-7))

    rna1 = rnorm(y1f)
    rna2 = rnorm(y2f)
    rnb1 = rnorm(z2f)
    rnb2 = rnorm(z1f)

    # feats[b, p, pr, t, cc, n] fp16, (pr,t): (0,0)=ay1 (0,1)=bz2 (1,0)=ay2 (1,1)=bz1
    feats = np.empty((B, 128, 2, 2, 2, N), np.float16)
    for (pr, t), a in (
        ((0, 0), y1f * rna1[:, None, :]),
        ((0, 1), z2f * rnb1[:, None, :]),
        ((1, 0), y2f * rna2[:, None, :]),
        ((1, 1), z1f * rnb2[:, None, :]),
    ):
        feats[:, :, pr, t] = (
            a.reshape(B, 2, 128, N).transpose(0, 2, 1, 3).astype(np.float16)
        )

    # small pack [p(112), b, 7*WW dyw | 28 dxt | 2 thr]
    SX = NT * WW + 28 + 2
    small = np.empty((B, TP, SX), np.float32)
    small[:, :, : NT * WW] = dyw.transpose(0, 1, 2, 3).reshape(B, TP, NT * WW)
    small[:, :, NT * WW : NT * WW + 28] = dxt
    small[:, :, NT * WW + 28 :] = np.broadcast_to(t2[:, None, :], (B, TP, 2))

    in_maps = []
    for c in range(NCORES):
        s = slice(c * BPC, (c + 1) * BPC)
        in_maps.append(
            {
                "feats": np.ascontiguousarray(feats[s]),
                "small": np.ascontiguousarray(small[s].transpose(1, 0, 2)),
                "woff": np.ascontiguousarray(woff[s].reshape(1, BPC * NT)),
            }
        )
    return in_maps, counts, WW


def kernel(y1, y2, z1, z2, view1_grid, view2_grid):
    y1 = np.asarray(y1, np.float32)
    y2 = np.asarray(y2, np.float32)
    z1 = np.asarray(z1, np.float32)
    z2 = np.asarray(z2, np.float32)
    view1_grid = np.asarray(view1_grid, np.float32)
    view2_grid = np.asarray(view2_grid, np.float32)

    in_maps, counts, WW = _prep_host(y1, y2, z1, z2, view1_grid, view2_grid)
    nc = _get_nc(WW)
    res = run_bass_kernel_spmd(nc, in_maps, core_ids=list(range(NCORES)))
    s = np.zeros(2, np.float64)
    for i in range(NCORES):
        o = res.results[i]["out"].astype(np.float64)  # [128, 2, BPC*NT]
        s += o.sum(axis=(0, 2))
    loss = -(
        np.float32(s[0]) / np.float32(counts[0])
        + np.float32(s[1]) / np.float32(counts[1])
    )
    return np.array(loss, dtype=np.float32)


# revision 6
# speedup vs baseline: 1.0014x; 1.0014x over previous
"""ConsistencyLoss kernel for 8 Trainium2 NeuronCores.

Math (per reference):
  For view1: sim = cos_sim_pairwise(y1, z2) [B,N,N]; mask from grid distances;
  loss_v = sum(sim*mask)/sum(mask); out = -(loss_1 + loss_2), N = 28*28 = 784.

Strategy (data-parallel over batch, 8 batches/core x 8 cores):
  Host prep:
    - Grids are separable: pairwise dist^2 D2[n,m] = Dy2[i(n),i'(m)] +
      Dx2[j(n),j'(m)] from two [28,28] tables per batch.
    - n tiled as 7x112 (4 image rows per tile, exact).  The mask band per
      tile fits in a WW-image-row window of m (WW computed from data,
      typically 6 -> 168 moving columns); host computes the window start per
      (batch, tile).
    - 1/||y_n|| folded into y features, 1/||z_m|| folded into z features
      (host, fp32), then features packed to fp16 in one dram tensor
      [BPC, 128, 2(view pair), 2(y|z), 2(c-half), N]; two DMAs per batch so
      view1 compute overlaps view2's feature transfer.
    - Mask counts (denominators) computed on host with bit-identical fp32
      add+compare to the device mask test.
  Device per (batch, tile):
    - Pool/GpSimd: d2 tile [112, WW*28] = dyw ⊕ dxt (broadcast add, SBUF).
    - PE: num_v = y_v^T @ z_v windowed (fp16 in, fp32 PSUM accum over the
      two 128-channel halves); window offset is a runtime PE register
      (bass.ds), 7 offsets loaded per batch in one TensorLoad.
    - DVE: scalar_tensor_tensor (d2 <= thr_v) * num_v with per-partition
      accumulation into ms[v][:, b*7+k].
    - ms accumulators DMA'd raw to HBM at the end (short tail); host does
      the final reduction.
  Host finish: sum the 8 cores' ms, divide by host counts.

The timeline is paced by the feature DMA stream (~36.4us of the ~45us
total); DVE masked-accumulate is the secondary leg (~34us), Pool builds
~24us, PE ~17us.
"""

import sys

sys.path.insert(0, "/opt/trn_rl_repo")

import numpy as np

import concourse.bass as bass
import concourse.mybir as mybir
import concourse.tile as tile
from concourse import bacc
from concourse.bass import broadcast_tensor_aps
from concourse.bass_utils import run_bass_kernel_spmd

B, C, H, W = 64, 256, 28, 28
N = H * W  # 784
NCORES = 8
BPC = B // NCORES  # batches per core
NT = 7  # n tiles of 112 partitions (4 image rows each)
TP = 112  # partitions per tile
THR = 0.7

F32 = mybir.dt.float32
F16 = mybir.dt.float16
I32 = mybir.dt.int32
ALU = mybir.AluOpType
ENG = mybir.EngineType

_COMPILED = {}


def _build_nc(WW=6):
    WWC = WW * 28  # window columns in m
    # tile groups sharing one PSUM bank (512 fp32 cols) and one stt each;
    # leftover group first so the tail's first stt fires early
    gmax = max(1, 512 // WWC)
    n_full = NT // gmax
    leftover = NT % gmax
    GROUPS = []
    if leftover:
        GROUPS.append(tuple(range(NT - leftover, NT)))
    for i in range(n_full):
        GROUPS.append(tuple(range(i * gmax, (i + 1) * gmax)))
    GROUPS = tuple(GROUPS)
    NG = len(GROUPS)
    nc = bacc.Bacc("TRN2", debug=False, num_devices=NCORES)

    # features: [b, p, pr, t, cc, n], pair pr0=(ay1,bz2), pr1=(ay2,bz1)
    feats = nc.dram_tensor("feats", [BPC, 128, 2, 2, 2, N], F16, kind="ExternalInput")
    # small pack: [p(112), b, 7*WW dyw | 28 dxt | 2 thr]
    SX = NT * WW + 28 + 2
    small = nc.dram_tensor("small", [TP, BPC, SX], F32, kind="ExternalInput")
    woff = nc.dram_tensor("woff", [1, BPC * NT], I32, kind="ExternalInput")
    out = nc.dram_tensor("out", [128, 2, BPC * NG], F32, kind="ExternalOutput")

    with tile.TileContext(nc) as tc:
        with (
            tc.tile_pool(name="feat", bufs=4) as feat_pool,
            tc.tile_pool(name="small", bufs=1) as sm_pool,
            tc.tile_pool(name="d2", bufs=2 * NG + 1) as d2_pool,
            tc.tile_pool(name="scr", bufs=3) as scr_pool,
            tc.tile_pool(name="accum", bufs=1) as acc_pool,
            tc.tile_pool(name="psum", bufs=5, space="PSUM") as psum_pool,
        ):
            fts = []
            for _ in range(2):
                ft = feat_pool.tile([128, 2, 2, 2, N], F16, tag="ft")
                fts.append(ft)
            nc.sync.dma_start(fts[0][:, 0, :, :, :], feats[0, :, 0])
            sm_t = sm_pool.tile([TP, BPC, SX], F32)
            nc.sync.dma_start(sm_t[:, :, :], small[:, :, :])
            woff_t = sm_pool.tile([1, BPC * NT], I32)
            nc.sync.dma_start(woff_t[:, :], woff[:, :])
            for cc in (0, 1):
                nc.sync.dma_start(fts[0][:, 1, :, cc, :], feats[0, :, 1, :, cc])
            nc.sync.dma_start(fts[1][:, 0, :, :, :], feats[1, :, 0])
            for cc in (0, 1):
                nc.sync.dma_start(fts[1][:, 1, :, cc, :], feats[1, :, 1, :, cc])

            ms = []
            for v in (0, 1):
                m = acc_pool.tile([128, BPC * NG], F32, tag=f"ms{v}")
                nc.vector.memset(m[:, :], 0.0)
                ms.append(m)

            regsets = [
                [nc.alloc_register(ENG.PE, f"w{s}_{k}") for k in range(NT)]
                for s in (0, 1)
            ]

            for b in range(BPC):
                if b < 2:
                    ft = fts[b]
                else:
                    ft = feat_pool.tile([128, 2, 2, 2, N], F16, tag="ft")
                    nc.sync.dma_start(ft[:, 0, :, :, :], feats[b, :, 0])
                    for cc in (0, 1):
                        nc.sync.dma_start(ft[:, 1, :, cc, :], feats[b, :, 1, :, cc])

                regs = regsets[b % 2]
                nc.tensor.load(regs, woff_t[0:1, b * NT : (b + 1) * NT])
                wvs = [
                    nc.snap(
                        bass.RegisterHandles([regs[k]]),
                        donate=True,
                        min_val=0,
                        max_val=(28 - WW) * 28,
                    )
                    for k in range(NT)
                ]

                d2s = []
                for g, ks in enumerate(GROUPS):
                    gw = len(ks)
                    d2 = d2_pool.tile([TP, gmax, WWC], F32, tag="d2")
                    i0, i1 = broadcast_tensor_aps(
                        sm_t[:, b, ks[0] * WW : (ks[0] + gw) * WW, None],
                        sm_t[:, b, None, NT * WW : NT * WW + 28],
                    )
                    nc.gpsimd.tensor_tensor(
                        d2[:, 0:gw, :].rearrange("q g (a c) -> q (g a) c", a=WW),
                        i0,
                        i1,
                        ALU.add,
                    )
                    d2s.append(d2)

                for v in (0, 1):
                    for g, ks in enumerate(GROUPS):
                        gw = len(ks)
                        num = psum_pool.tile([TP, gmax, WWC], F32, tag="num")
                        for j, k in enumerate(ks):
                            for cc in (0, 1):
                                nc.tensor.matmul(
                                    num[:, j, :],
                                    ft[:, v, 0, cc, k * TP : (k + 1) * TP],
                                    ft[:, v, 1, cc, bass.ds(wvs[k], WWC)],
                                    start=(cc == 0),
                                    stop=(cc == 1),
                                )
                        scr = scr_pool.tile([TP, gmax * WWC], F32, tag="scr")
                        col = b * NG + g
                        nc.vector.scalar_tensor_tensor(
                            out=scr[:, 0 : gw * WWC],
                            in0=d2s[g][:, 0:gw, :],
                            scalar=sm_t[:, b, NT * WW + 28 + v : NT * WW + 29 + v],
                            in1=num[:, 0:gw, :],
                            op0=ALU.is_le,
                            op1=ALU.mult,
                            accum_out=ms[v][0:TP, col : col + 1],
                        )

            nc.scalar.dma_start(out[:, 0, :], ms[0][:, :])
            nc.sync.dma_start(
                out[:, 1, 0 : (BPC - 1) * NG], ms[1][:, 0 : (BPC - 1) * NG]
            )
            nc.sync.dma_start(
                out[:, 1, (BPC - 1) * NG :], ms[1][:, (BPC - 1) * NG :]
            )

    nc.compile()
    return nc


def _get_nc(WW):
    if WW not in _COMPILED:
        _COMPILED[WW] = _build_nc(WW)
    return _COMPILED[WW]


def _prep_host(y1, y2, z1, z2, view1_grid, view2_grid):
    """Host-side prep: separable distance tables, norms, counts, shards."""
    y1f = y1.reshape(B, C, N)
    y2f = y2.reshape(B, C, N)
    z1f = z1.reshape(B, C, N)
    z2f = z2.reshape(B, C, N)

    # --- separable grid tables ------------------------------------------
    g1y = view1_grid[:, 0, :, 0]  # [B, 28]
    g1x = view1_grid[:, 1, 0, :]
    g2y = view2_grid[:, 0, :, 0]
    g2x = view2_grid[:, 1, 0, :]
    if not (
        np.array_equal(view1_grid[:, 0], np.broadcast_to(g1y[:, :, None], (B, H, W)))
        and np.array_equal(view1_grid[:, 1], np.broadcast_to(g1x[:, None, :], (B, H, W)))
        and np.array_equal(view2_grid[:, 0], np.broadcast_to(g2y[:, :, None], (B, H, W)))
        and np.array_equal(view2_grid[:, 1], np.broadcast_to(g2x[:, None, :], (B, H, W)))
    ):
        raise RuntimeError("grids are not separable; unsupported input")

    dy = g1y[:, :, None] - g2y[:, None, :]  # fp32 [B,28,28]
    dx = g1x[:, :, None] - g2x[:, None, :]
    dy2 = dy * dy
    dx2 = dx * dx

    v1bin = np.linalg.norm(view1_grid[..., 1, 1] - view1_grid[..., 0, 0], axis=-1)
    v2bin = np.linalg.norm(view2_grid[..., 1, 1] - view2_grid[..., 0, 0], axis=-1)
    t2 = np.empty((B, 2), np.float32)
    t2[:, 0] = ((THR * v1bin.astype(np.float64)) ** 2).astype(np.float32)
    t2[:, 1] = ((THR * v2bin.astype(np.float64)) ** 2).astype(np.float32)

    # --- per-(batch, tile) windows of valid i' --------------------------
    tmax2 = np.maximum(t2[:, 0], t2[:, 1]).astype(np.float64) * (1 + 1e-6)  # [B]
    w0 = np.zeros((B, NT), np.int32)
    widths = np.zeros((B, NT), np.int64)
    for k in range(NT):
        sub_min = dy2[:, 4 * k : 4 * k + 4, :].min(axis=1)  # [B, 28]
        valid = sub_min <= tmax2[:, None]
        any_valid = valid.any(axis=1)
        first = np.argmax(valid, axis=1)
        last = 27 - np.argmax(valid[:, ::-1], axis=1)
        widths[:, k] = np.where(any_valid, last - first + 1, 1)
        w0[:, k] = np.where(any_valid, first, 0)
    WW = max(6, int(widths.max()))
    if WW > 28:
        raise RuntimeError("mask window exceeds image; unsupported input")
    w0 = np.minimum(w0, 28 - WW).astype(np.int32)

    # dyw[b, p(112), k, a] = dy2[b, 4k + p//28, w0[b,k]+a]
    iidx = np.arange(TP) // 28  # [112] image row within tile
    cols = w0[:, :, None] + np.arange(WW)[None, None, :]  # [B, NT, WW]
    dyw = dy2[
        np.arange(B)[:, None, None, None],
        (iidx[None, :, None, None] + 4 * np.arange(NT)[None, None, :, None]),
        cols[:, None, :, :],
    ]  # [B, 112, NT, WW]
    woff = (w0 * 28).astype(np.int32).reshape(B, NT)

    # dxt[b, p, c] = dx2[b, p%28, c]
    dxt = dx2[:, np.tile(np.arange(28), 4), :]  # [B, 112, 28]

    # --- mask counts (bit-identical fp32 add + compare as device) -------
    counts = np.zeros(2, np.int64)
    for b in range(B):
        d2b = dy2[b][:, None, :, None] + dx2[b][None, :, None, :]  # fp32
        counts[0] += int((d2b <= t2[b, 0]).sum())
        counts[1] += int((d2b <= t2[b, 1]).sum())

    # --- norms (both sides folded on host) ------------------------------
    def rnorm(a):
        n = np.sqrt(np.einsum("bcn,bcn->bn", a, a, dtype=np.float32))
        return 1.0 / np.maximum(n, np.float32(1e-7))

    rna1 = rnorm(y1f)
    rna2 = rnorm(y2f)
    rnb1 = rnorm(z2f)
    rnb2 = rnorm(z1f)

    # feats[b, p, pr, t, cc, n] fp16, (pr,t): (0,0)=ay1 (0,1)=bz2 (1,0)=ay2 (1,1)=bz1
    feats = np.empty((B, 128, 2, 2, 2, N), np.float16)
    for (pr, t), a in (
        ((0, 0), y1f * rna1[:, None, :]),
        ((0, 1), z2f * rnb1[:, None, :]),
        ((1, 0), y2f * rna2[:, None, :]),
        ((1, 1), z1f * rnb2[:, None, :]),
    ):
        feats[:, :, pr, t] = (
            a.reshape(B, 2, 128, N).transpose(0, 2, 1, 3).astype(np.float16)
        )

    # small pack [p(112), b, 7*WW dyw | 28 dxt | 2 thr]
    SX = NT * WW + 28 + 2
    small = np.empty((B, TP, SX), np.float32)
    small[:, :, : NT * WW] = dyw.transpose(0, 1, 2, 3).reshape(B, TP, NT * WW)
    small[:, :, NT * WW : NT * WW + 28] = dxt
    small[:, :, NT * WW + 28 :] = np.broadcast_to(t2[:, None, :], (B, TP, 2))

    in_maps = []
    for c in range(NCORES):
        s = slice(c * BPC, (c + 1) * BPC)
        in_maps.append(
            {
                "feats": np.ascontiguousarray(feats[s]),
                "small": np.ascontiguousarray(small[s].transpose(1, 0, 2)),
                "woff": np.ascontiguousarray(woff[s].reshape(1, BPC * NT)),
            }
        )
    return in_maps, counts, WW


def kernel(y1, y2, z1, z2, view1_grid, view2_grid):
    y1 = np.asarray(y1, np.float32)
    y2 = np.asarray(y2, np.float32)
    z1 = np.asarray(z1, np.float32)
    z2 = np.asarray(z2, np.float32)
    view1_grid = np.asarray(view1_grid, np.float32)
    view2_grid = np.asarray(view2_grid, np.float32)

    in_maps, counts, WW = _prep_host(y1, y2, z1, z2, view1_grid, view2_grid)
    nc = _get_nc(WW)
    res = run_bass_kernel_spmd(nc, in_maps, core_ids=list(range(NCORES)))
    s = np.zeros(2, np.float64)
    for i in range(NCORES):
        o = res.results[i]["out"].astype(np.float64)  # [128, 2, BPC*NT]
        s += o.sum(axis=(0, 2))
    loss = -(
        np.float32(s[0]) / np.float32(counts[0])
        + np.float32(s[1]) / np.float32(counts[1])
    )
    return np.array(loss, dtype=np.float32)
